# revision 31
# baseline (speedup 1.0000x reference)
"""Trainium2 Bass kernel for nn_GCN_23029614641773.

The reference GCN operates on B independent 27-node graphs where every node of
graph i starts with the same feature vector x[i], and only node 0 of each graph
feeds the classifier head. Exploiting linearity of the edge aggregation, the
whole network collapses exactly (up to fp rounding order) to a per-sample MLP:

    y = x @ W0                                  # [B, 1024]
    s = lrelu(y + b0) + 2*lrelu(3y + b0) + lrelu(5y + b0)
      # node 1's in-neighbours {0,2,4,6} have in-degrees {1,3,3,5};
      # 2*lrelu(3y+b0) == lrelu(6y+2*b0) exactly (scaling by 2 is exact).
      # With b0 == 0 (spec fill): s == max(12y, 2.4y) exactly.
    t = s @ W1;  h = lrelu(t + b1)              # [B, 512]
    v = h @ W2;  g = lrelu(v + b2)              # [B, 256]
    out = g @ Wc + bc                           # [B, 1]

Sharding: pure data parallelism, batch split across 8 NeuronCores; each core
holds the full weight set.

Perf design (measured on HW, ~43us baseline -> ~30us):
- fp16 everywhere (host-cast): halves HBM traffic AND halves PE cost vs
  fp32r (fp16 matmul+weight-load ~160ns vs 213ns per [128,128]x[128,256]).
  Measured end-to-end rel err ~8e-4; every fp8 variant exceeds the 2e-2
  gate, so fp8 is out.
- Host pre-packs every tensor into exact SBUF tile layout (x arrives
  pre-transposed; weights as [128, m-block x k x 128]) so every DMA is a
  flat [128, N] copy: one descriptor per partition. This matters because
  HWDGE descriptor generation costs ~650ns per dma_start on the issuing
  sequencer regardless of size — few, large DMAs win (22 DMAs cost 15us
  of serialized descriptor-gen; 8 cost 5us).
- ALL bulk loads on ONE queue (sync ring), ordered by consumption time:
  SDMA engines round-robin active queues at packet granularity, so a
  second queue halves throughput and lets late-needed data cut in line.
- x^T+W0(m0) ride in ONE leading DMA (a DMA's completion receipt costs
  ~1.7us before dependent compute dispatches, so the first-matmul gate
  pays it once); remaining W0 is chunked (m1 | m2-3 | m4-5 | m6-7) to
  stay just ahead of the PE's ~1us-per-m-block L1 cadence.
- PE warmup: ~20 dummy matmuls fill the input-DMA wait so the tensor
  engine's ramping clock is at full speed when real data lands (cold
  matmuls run 2.5-4x slower; an idle gap >~1us re-cools the array, so
  the warmup must run contiguously into the real work).
- The last ~10.5us is a fixed floor: ~0.7us out-DMA descriptor gen +
  ~1.6us HBM write receipt + ~8.2us NEFF epilogue (constant after the
  last DMA completes — measured identical for a trivial kernel).
"""

from contextlib import ExitStack

import numpy as np

import concourse.bacc as bacc
import concourse.mybir as mybir
import concourse.tile as tile
from concourse.bass_utils import run_bass_kernel_spmd

F32 = mybir.dt.float32
F16 = mybir.dt.float16
P = 128
N_CORES = 8
B_FULL = 2048
B = B_FULL // N_CORES  # 256 rows per core
D0, D1, D2, D3 = 1024, 1024, 512, 256
K0, M0 = D0 // P, D1 // P  # 8, 8
K1, M1 = D1 // P, D2 // P  # 8, 4
K2, M2 = D2 // P, D3 // P  # 4, 2
KC = D3 // P  # 2

NEG_SLOPE = 0.2


def _build(zero_bias: bool):
    nc = bacc.Bacc(
        "TRN2", target_bir_lowering=False, debug=False,
        enable_asserts=False, num_devices=1,
    )

    # x^T and W0's m-block 0 ride in one tensor: one DMA (one completion
    # receipt) gates the first matmul group.
    xw_d = nc.dram_tensor("xw", [P, K0 * B + K0 * P], F16,
                          kind="ExternalInput").ap()
    w0_d = nc.dram_tensor("w0r", [P, (M0 - 1) * K0 * P], F16,
                          kind="ExternalInput").ap()
    w1_d = nc.dram_tensor("w1p", [P, M1 * K1 * P], F16, kind="ExternalInput").ap()
    # W2 blocks and Wc ride in one packed tensor (one DMA, one tile).
    w2_d = nc.dram_tensor("w2p", [P, M2 * K2 * P + KC], F16,
                          kind="ExternalInput").ap()
    if not zero_bias:
        b0_d = nc.dram_tensor("b0", [D1], F32, kind="ExternalInput").ap()
        b1_d = nc.dram_tensor("b1", [D2], F32, kind="ExternalInput").ap()
        b2_d = nc.dram_tensor("b2", [D3], F32, kind="ExternalInput").ap()
        bc_d = nc.dram_tensor("bc", [1], F32, kind="ExternalInput").ap()
    out_d = nc.dram_tensor("out", [1, B], F32, kind="ExternalOutput").ap()

    with ExitStack() as ctx:
        tc = ctx.enter_context(tile.TileContext(nc))
        const = ctx.enter_context(tc.tile_pool(name="const", bufs=1))
        xt_p = ctx.enter_context(tc.tile_pool(name="xt", bufs=1))
        w0_p = ctx.enter_context(tc.tile_pool(name="w0", bufs=5))
        w1_p = ctx.enter_context(tc.tile_pool(name="w1", bufs=1))
        w2_p = ctx.enter_context(tc.tile_pool(name="w2", bufs=1))
        s_p = ctx.enter_context(tc.tile_pool(name="s", bufs=K1))
        h_p = ctx.enter_context(tc.tile_pool(name="h", bufs=K2))
        g_p = ctx.enter_context(tc.tile_pool(name="g", bufs=KC))
        tmp_p = ctx.enter_context(tc.tile_pool(name="tmp", bufs=4))
        out_p = ctx.enter_context(tc.tile_pool(name="outp", bufs=1))
        ps_p = ctx.enter_context(tc.tile_pool(name="ps", bufs=6, space="PSUM"))
        cls_ps = ctx.enter_context(tc.tile_pool(name="cls", bufs=1, space="PSUM"))
        warm_ps = ctx.enter_context(tc.tile_pool(name="warm", bufs=1,
                                                 space="PSUM"))

        # leaky-relu slope as a per-partition alpha vector for ACT Prelu
        alt = const.tile([P, 1], F32, tag="alt")
        nc.vector.memset(alt[:], NEG_SLOPE)

        # ---- PE warmup: the tensor engine's clock ramps with sustained use
        # (~2x slower cold). Fill the otherwise-idle input-DMA window with a
        # zero matmul accumulation group so the array is at full p-state when
        # real data lands. One group -> no inter-matmul semaphores. Sized to
        # end ~when the first input DMA's completion fires; it must run
        # CONTIGUOUSLY into the real work — an idle gap lets the clock drop
        # again (measured: a 2us gap re-cooled the whole L1). ----
        NWARM = 20
        wz = const.tile([P, B], F16, tag="wz")
        nc.vector.memset(wz[:], 0.0)
        pw = warm_ps.tile([P, B], F32)
        for i in range(NWARM):
            nc.tensor.matmul(pw[:], lhsT=wz[:, 0:P], rhs=wz[:],
                             start=(i == 0), stop=(i == NWARM - 1))

        # ---- DMA plan. Two HW facts drive this (measured from traces):
        # (1) HWDGE descriptor generation costs ~650ns per dma_start
        #     (128 per-partition descriptors @ ~5ns), serialized on the
        #     issuing sequencer, INDEPENDENT of transfer size.
        # (2) The 16 SDMA engines round-robin between ACTIVE QUEUES at
        #     packet granularity — concurrent queues halve per-engine
        #     throughput and let late-needed data cut in line.
        # So: ONE queue (sync HWDGE) for all bulk data, few large DMAs,
        # strictly ordered by when compute needs them. W0 is split so
        # m-block 0 lands early (PE start) while the rest stream behind
        # the L1 m-loop. ----
        xw_t = xt_p.tile([P, K0 * B + K0 * P], F16, tag="xt", name="xw")
        nc.sync.dma_start(xw_t[:], xw_d)
        xt = [xw_t[:, k * B:(k + 1) * B] for k in range(K0)]
        w0m0 = xw_t[:, K0 * B:K0 * B + K0 * P]

        # Remaining W0 m-blocks, chunked to stay just ahead of the PE's
        # ~1us-per-m-block L1 cadence given ~1.7us DMA completion latency.
        w0rest = []
        for lo, hi in ((1, 2), (2, 4), (4, 6), (6, 8)):
            t = w0_p.tile([P, (hi - lo) * K0 * P], F16, tag="w",
                          name=f"w0_{lo}_{hi}")
            nc.sync.dma_start(
                t[:], w0_d[:, (lo - 1) * K0 * P:(hi - 1) * K0 * P])
            w0rest.append((lo, hi, t))
        w1t = w1_p.tile([P, M1 * K1 * P], F16, tag="w", name="w1")
        nc.sync.dma_start(w1t[:], w1_d)
        w2t = w2_p.tile([P, M2 * K2 * P + KC], F16, tag="w", name="w2")
        nc.sync.dma_start(w2t[:], w2_d)
        wc = w2t[:, M2 * K2 * P:M2 * K2 * P + KC]

        def w0_lhsT(m, k):
            if m == 0:
                return w0m0[:, k * P:(k + 1) * P]
            for lo, hi, t in w0rest:
                if lo <= m < hi:
                    off = ((m - lo) * K0 + k) * P
                    return t[:, off:off + P]
            raise AssertionError(m)

        def w1_lhsT(m, k):
            return w1t[:, (m * K1 + k) * P:(m * K1 + k + 1) * P]

        def w2_lhsT(m, k):
            return w2t[:, (m * K2 + k) * P:(m * K2 + k + 1) * P]

        if not zero_bias:
            b0t = const.tile([P, M0], F32, tag="b0t")
            nc.scalar.dma_start(b0t[:], b0_d.rearrange("(c p) -> p c", p=P))
            b1t = const.tile([P, M1], F32, tag="b1t")
            nc.scalar.dma_start(b1t[:], b1_d.rearrange("(c p) -> p c", p=P))
            b2t = const.tile([P, M2], F32, tag="b2t")
            nc.scalar.dma_start(b2t[:], b2_d.rearrange("(c p) -> p c", p=P))
            bct = const.tile([1, 1], F32, tag="bct")
            nc.scalar.dma_start(bct[:], bc_d.rearrange("(a b) -> a b", a=1))
            b0t2 = const.tile([P, M0], F32, tag="b0t2")
            nc.vector.tensor_scalar_mul(b0t2[:], b0t[:], 2.0)

        PRELU = mybir.ActivationFunctionType.Prelu

        def matmul_group(ps, lhsT_fn, m, rhs_tiles, K):
            for k in range(K):
                nc.tensor.matmul(
                    ps[:], lhsT=lhsT_fn(m, k),
                    rhs=rhs_tiles[k],
                    start=(k == 0), stop=(k == K - 1),
                )

        # ---- layer 1: y[m] = sum_k W0[k,m].T @ xT[k];
        #      s = 12*lrelu(y) = Prelu(12*y) exactly (zero bias) ----
        s_tiles = []
        for m in range(M0):
            ps = ps_p.tile([P, B], F32, tag="ps", name=f"ps1_{m}")
            matmul_group(ps, w0_lhsT, m, xt, K0)
            s = s_p.tile([P, B], F16, tag="s", name=f"s_{m}")
            if zero_bias:
                nc.scalar.activation(s[:], ps[:], PRELU, scale=12.0, alpha=alt[:])
            else:
                acc = tmp_p.tile([P, B], F32, tag="l", name=f"acc_{m}")
                first = True
                for scale, bias in ((1.0, b0t[:, m:m + 1]), (6.0, b0t2[:, m:m + 1]),
                                    (5.0, b0t[:, m:m + 1])):
                    l = tmp_p.tile([P, B], F32, tag="l", name=f"l_{m}")
                    nc.scalar.activation(l[:], ps[:], PRELU,
                                         scale=scale, bias=bias, alpha=alt[:])
                    if first:
                        nc.vector.tensor_copy(acc[:], l[:])
                        first = False
                    else:
                        nc.vector.tensor_add(acc[:], acc[:], l[:])
                nc.vector.tensor_copy(s[:], acc[:])
            s_tiles.append(s)

        # ---- layer 2: t[m] = sum_k W1[k,m].T @ s[k]; h = lrelu(t + b1) ----
        h_tiles = []
        for m in range(M1):
            ps = ps_p.tile([P, B], F32, tag="ps", name=f"ps2_{m}")
            matmul_group(ps, w1_lhsT, m, [t[:] for t in s_tiles], K1)
            h = h_p.tile([P, B], F16, tag="h", name=f"h_{m}")
            if zero_bias:
                nc.scalar.activation(h[:], ps[:], PRELU, alpha=alt[:])
            else:
                nc.scalar.activation(h[:], ps[:], PRELU,
                                     bias=b1t[:, m:m + 1], alpha=alt[:])
            h_tiles.append(h)

        # ---- layer 3: v[m] = sum_k W2[k,m].T @ h[k]; g = lrelu(v + b2),
        # with the classifier matmuls (out[1,B] = sum_c Wc[c].T @ g[c])
        # interleaved so cls c=0 runs while g1's activation completes ----
        g_tiles = []
        po = cls_ps.tile([1, B], F32)
        for m in range(M2):
            ps = ps_p.tile([P, B], F32, tag="ps", name=f"ps3_{m}")
            matmul_group(ps, w2_lhsT, m, [t[:] for t in h_tiles], K2)
            g = g_p.tile([P, B], F16, tag="g", name=f"g_{m}")
            if zero_bias:
                nc.scalar.activation(g[:], ps[:], PRELU, alpha=alt[:])
            else:
                nc.scalar.activation(g[:], ps[:], PRELU,
                                     bias=b2t[:, m:m + 1], alpha=alt[:])
            g_tiles.append(g)
        for c in range(KC):
            nc.tensor.matmul(
                po[:], lhsT=wc[:, c:c + 1], rhs=g_tiles[c][:],
                start=(c == 0), stop=(c == KC - 1),
            )
        ob = out_p.tile([1, B], F32)
        if zero_bias:
            nc.vector.tensor_copy(ob[:], po[:])
        else:
            nc.vector.tensor_scalar_add(ob[:], po[:], bct[:, 0:1])
        nc.sync.dma_start(out_d, ob[:])

    nc.compile()
    return nc


_CACHE = {}


def _get_nc(zero_bias: bool):
    if zero_bias not in _CACHE:
        _CACHE[zero_bias] = _build(zero_bias)
    return _CACHE[zero_bias]


def _run(inputs, trace=False, **kw):
    def f32(a):
        return np.ascontiguousarray(np.asarray(a), dtype=np.float32)

    x = f32(inputs["x"])
    W0, b0 = f32(inputs["W0"]), f32(inputs["b0"])
    W1, b1 = f32(inputs["W1"]), f32(inputs["b1"])
    W2, b2 = f32(inputs["W2"]), f32(inputs["b2"])
    Wc, bc = f32(inputs["Wc"]), f32(inputs["bc"])
    zero_bias = not (b0.any() or b1.any() or b2.any() or bc.any())
    nc = _get_nc(zero_bias)

    # Host-side packing into exact SBUF tile layouts, fp16.
    def pack_w(W, K, M):
        return np.ascontiguousarray(
            W.astype(np.float16).reshape(K, P, M, P)
            .transpose(1, 2, 0, 3).reshape(P, M * K * P))

    w0p = pack_w(W0, K0, M0)
    w0r = np.ascontiguousarray(w0p[:, K0 * P:])
    w1p = pack_w(W1, K1, M1)
    wcp = Wc.astype(np.float16)[:, 0].reshape(KC, P).T  # [128, 2]
    w2p = np.ascontiguousarray(
        np.concatenate([pack_w(W2, K2, M2), wcp], axis=1))

    in_maps = []
    for i in range(N_CORES):
        xs = x[i * B:(i + 1) * B].astype(np.float16)  # [256, 1024]
        xtp = xs.T.reshape(K0, P, B).transpose(1, 0, 2).reshape(P, K0 * B)
        xwp = np.ascontiguousarray(
            np.concatenate([xtp, w0p[:, 0:K0 * P]], axis=1))
        m = {"xw": xwp, "w0r": w0r, "w1p": w1p, "w2p": w2p}
        if not zero_bias:
            m.update({"b0": b0, "b1": b1, "b2": b2, "bc": bc})
        in_maps.append(m)
    res = run_bass_kernel_spmd(nc, in_maps, list(range(N_CORES)),
                               trace=trace, **kw)
    out = np.empty((B_FULL, 1), dtype=np.float32)
    for i in range(N_CORES):
        out[i * B:(i + 1) * B, 0] = res.results[i]["out"][0]
    return out, res


def kernel(**inputs) -> np.ndarray:
    out, _ = _run(inputs)
    return out
